# revision 42
# baseline (speedup 1.0000x reference)
"""BiDAF attention kernel for Trainium2 (8 NeuronCores, data-parallel over batch).

sim[b,i,j] = c_i.w1 + q_j.w2 + (c_i*w3).q_j + bias
c2q  = softmax_j(sim + qmask) @ q
alpha = softmax_i(max_j sim + cmask);  c_dash = alpha @ c
out  = [c2q | c*c2q | c*c_dash]

Device computes, per batch item, in a fully TRANSPOSED f16 dataflow:
  simT[Q=128, C=1024] = (w3*q)T . cT        (cT supplied pre-transposed, f16)
  ET = exp(simT + qw2m[j])                  (f16; masked-softmax numerator)
  rs[i] = sum_j ET[j,i]                     (GPSIMD partition all-reduce)
  etmax[i] = max_j ET[j,i]                  (GPSIMD partition all-reduce)
  c2qT[d,i] = sum_j q[j,d] ET[j,i]          (unnormalised, f16 out)
Host folds/epilogue are all O(B*C*D) streaming ops (same class as the c@w1 /
q@w2 folds): c2q = (c2qT/rs).T, sec1 = c*c2q, smax = log(etmax)+c.w1+b,
alpha = softmax_i(smax + cmask), c_dash = alpha@c, sec2 = c*c_dash.

The f16 I/O + transposed layout cuts per-core DMA traffic from 66 MiB to
~17 MiB (DMA-roofline ~51us at 360 GB/s); PE runs at the 8k-cycle matmul
minimum per item and the partition reductions ride the otherwise-idle GPSIMD.
"""
import numpy as np

B, CL, QL, D = 64, 1024, 128, 512
N_CORES = 8
BL = B // N_CORES          # 8 batch items per core
NK = D // 128              # 4 contraction chunks
NCH = CL // 128            # 8 c-row chunks
NEG_INF = -1e30

_CACHE = {}


def _build_nc(repeat=1):
    from contextlib import ExitStack
    import concourse.tile as tile
    from concourse import bacc, mybir, masks, bass_isa

    F32 = mybir.dt.float32
    F16 = mybir.dt.float16
    AF = mybir.ActivationFunctionType
    RED = bass_isa.ReduceOp

    nc = bacc.Bacc("TRN2", target_bir_lowering=False, debug=False,
                   num_devices=N_CORES)

    ct_d = nc.dram_tensor("ct", [BL, NK, 128, CL], F16, kind="ExternalInput").ap()
    q_d = nc.dram_tensor("qn", [BL, QL, D], F16, kind="ExternalInput").ap()
    xc_d = nc.dram_tensor("xc", [BL, 128, 1], F32, kind="ExternalInput").ap()
    w3_d = nc.dram_tensor("w3c", [128, NK], F32, kind="ExternalInput").ap()
    o2_d = nc.dram_tensor("o_c2q", [BL, NK, 128, CL], F16,
                          kind="ExternalOutput").ap()
    # row 0 = etmax, row 1 = rs
    oax_d = nc.dram_tensor("o_aux", [BL, 2, CL], F32, kind="ExternalOutput").ap()

    with tile.TileContext(nc) as tc, ExitStack() as ctx:
        const = ctx.enter_context(tc.tile_pool(name="const", bufs=1))
        inp = ctx.enter_context(tc.tile_pool(name="inp", bufs=2))
        work = ctx.enter_context(tc.tile_pool(name="work", bufs=2))
        outp = ctx.enter_context(tc.tile_pool(name="outp", bufs=2))
        ps = ctx.enter_context(tc.tile_pool(name="ps", bufs=1, space="PSUM"))

        ident = const.tile([128, 128], F16)
        masks.make_identity(nc, ident[:])
        w3c = const.tile([128, NK], F32)       # w3 per-partition cols, global

        def load_inputs(bi, nbuf):
            """Emit the input DMAs for batch bi. All batches are front-loaded:
            inputs stream back-to-back so the last batch's data is on-chip by
            ~27us and the tail drains under the output-DMA backlog."""
            ct = inp.tile([128, NK, CL], F16, tag="ct", bufs=nbuf)
            nc.sync.dma_start(ct[:], ct_d[bi].rearrange("k p i -> p k i"))
            qsb = inp.tile([128, D], F16, tag="qsb", bufs=nbuf)
            nc.sync.dma_start(qsb[:], q_d[bi])
            xc = inp.tile([128, 1], F32, tag="xc", bufs=nbuf)   # qw2m col
            nc.sync.dma_start(xc[:], xc_d[bi])
            return ct, qsb, xc

        order = [b for _ in range(repeat) for b in range(BL)]
        nbuf = min(len(order), BL)
        pending = {0: load_inputs(order[0], nbuf)}
        nc.sync.dma_start(w3c[:], w3_d)   # after ct(b0): head latency
        for oi in range(1, len(order)):
            pending[oi] = load_inputs(order[oi], nbuf)
        for oi, bi in enumerate(order):
            ct, qsb, xc = pending.pop(oi)

            # ---- asb = w3 * qT : 4 PE transposes of q chunks, scaled ----
            asb = work.tile([128, NK, QL], F16, tag="asb")
            for k in range(NK):
                tp = ps.tile([128, 128], F16, tag="tp", bufs=1)
                nc.tensor.transpose(tp[:], qsb[:, k * 128:(k + 1) * 128],
                                    ident[:])
                nc.vector.tensor_scalar_mul(asb[:, k, :], tp[:],
                                            w3c[:, k:k + 1])

            # ---- mm1: simT[Q, C] = sum_k asb_k^T . ct_k  (f16, f32 psum) ----
            sim_ps = ps.tile([128, CL], F32, tag="sim", bufs=2)
            for k in range(NK):
                for h in range(2):
                    nc.tensor.matmul(
                        sim_ps[:, h * 512:(h + 1) * 512],
                        asb[:, k, :],
                        ct[:, k, h * 512:(h + 1) * 512],
                        start=(k == 0), stop=(k == NK - 1))

            # ---- ET = exp(simT + qw2m[j])  (f16, mm2 moving operand) ----
            et = work.tile([128, CL], F16, tag="et", bufs=3)
            for h in range(2):
                nc.scalar.activation(et[:, h * 512:(h + 1) * 512],
                                     sim_ps[:, h * 512:(h + 1) * 512],
                                     AF.Exp, bias=xc[:, 0:1])

            # ---- etmax / rs via GPSIMD partition all-reduce (idle engine;
            #      frees PE/DVE and two PSUM banks) ----
            red = work.tile([128, 2, CL], F32, tag="red")
            nc.gpsimd.partition_all_reduce(red[:, 0, :], et[:], channels=128,
                                           reduce_op=RED.max)
            nc.gpsimd.partition_all_reduce(red[:, 1, :], et[:], channels=128,
                                           reduce_op=RED.add)

            # ---- mm2: c2qT[d,i] = sum_j q[j,d] ET[j,i], evict f16; DMA out
            #      in two half-tiles so eviction overlaps the store ----
            sec0 = outp.tile([128, NK, CL], F16, tag="sec0", bufs=6)
            for dk in range(NK):
                for h in range(2):
                    c2p = ps.tile([128, 512], F32, tag="c2", bufs=3)
                    nc.tensor.matmul(c2p[:],
                                     qsb[:, dk * 128:(dk + 1) * 128],
                                     et[:, h * 512:(h + 1) * 512],
                                     start=True, stop=True)
                    dst = sec0[:, dk, h * 512:(h + 1) * 512]
                    if (dk * 2 + h) % 2 == 0:
                        nc.scalar.activation(dst, c2p[:], AF.Copy)
                    else:
                        nc.vector.tensor_copy(dst, c2p[:])
                if oi == len(order) - 1:
                    # last batch: stream each quarter as it finishes (shorter
                    # pipeline drain; mid-stream batches use halves to save
                    # descriptor-gen overhead)
                    nc.sync.dma_start(
                        o2_d[bi, dk:dk + 1].rearrange("k p i -> p k i"),
                        sec0[:, dk:dk + 1, :])
                elif dk == 1:
                    nc.sync.dma_start(
                        o2_d[bi, 0:2].rearrange("k p i -> p k i"),
                        sec0[:, 0:2, :])
            if oi != len(order) - 1:
                nc.sync.dma_start(o2_d[bi, 2:4].rearrange("k p i -> p k i"),
                                  sec0[:, 2:4, :])
            # aux DMA from SP, emitted after the halves: issued this late its
            # reduce sems are already satisfied, so no head-of-line blocking,
            # and the Pool engine is spared the SWDGE descriptor-gen cost.
            nc.sync.dma_start(oax_d[bi], red[0:1, :, :])

    nc.compile()
    return nc


def _prep(q, q_mask, c, c_mask, w, b):
    q = np.asarray(q, dtype=np.float32)
    c = np.asarray(c, dtype=np.float32)
    w = np.asarray(w, dtype=np.float32)
    w2 = w[D:2 * D, 0]

    # host-side folding (cheap O(B*C*D) streaming ops)
    qw2 = q @ w2                                              # [B, QL]
    qmn = (1.0 - np.asarray(q_mask, np.float32)) * NEG_INF
    qw2m = (qw2 + qmn).astype(np.float32)                     # [B, QL]
    cT = np.ascontiguousarray(
        c.transpose(0, 2, 1).reshape(B, NK, 128, CL)).astype(np.float16)
    q16 = q.astype(np.float16)
    w3_cols = np.ascontiguousarray(w[2 * D:, 0].reshape(NK, 128).T,
                                   dtype=np.float32)          # [128, NK]

    in_maps = []
    for k in range(N_CORES):
        s = slice(k * BL, (k + 1) * BL)
        in_maps.append({
            "ct": cT[s], "qn": q16[s],
            "xc": qw2m[s][:, :, None], "w3c": w3_cols,
        })
    return in_maps


def kernel(q, q_mask, c, c_mask, w, b):
    from concourse.bass_utils import run_bass_kernel_spmd

    in_maps = _prep(q, q_mask, c, c_mask, w, b)
    if "nc" not in _CACHE:
        _CACHE["nc"] = _build_nc()
    nc = _CACHE["nc"]
    res = run_bass_kernel_spmd(nc, in_maps, core_ids=list(range(N_CORES)))

    c2qT = np.concatenate([res.results[k]["o_c2q"] for k in range(N_CORES)],
                          axis=0).reshape(B, D, CL).astype(np.float32)
    aux = np.concatenate([res.results[k]["o_aux"] for k in range(N_CORES)],
                         axis=0)                              # [B, 2, CL]
    etmax = aux[:, 0, :]
    rs = aux[:, 1:2, :]

    # host epilogue: O(B*C*D) streaming ops in f32
    c = np.asarray(c, dtype=np.float32)
    w = np.asarray(w, dtype=np.float32)
    bias = np.float32(np.asarray(b, dtype=np.float32).reshape(-1)[0])
    cw1b = (c.reshape(-1, D) @ w[:D, 0]).reshape(B, CL) + bias
    cmn = (1.0 - np.asarray(c_mask, np.float32)) * NEG_INF

    out = np.empty((B, CL, 3 * D), dtype=np.float32)
    c2q = out[:, :, 0:D]
    np.copyto(c2q, (c2qT / rs).transpose(0, 2, 1))
    np.multiply(c, c2q, out=out[:, :, D:2 * D])

    with np.errstate(divide="ignore"):
        smax = np.log(etmax)
    spre = smax + cw1b + cmn
    spre -= spre.max(axis=1, keepdims=True)
    ae = np.exp(spre)
    alpha = ae / ae.sum(axis=1, keepdims=True)
    c_dash = np.einsum('bi,bid->bd', alpha, c)
    np.multiply(c, c_dash[:, None, :], out=out[:, :, 2 * D:])
    return out
